# revision 28
# baseline (speedup 1.0000x reference)
"""Trainium2 Bass kernel for head_dim==1 cross-attention + out-projection.

Problem (hardcoded shapes):
  query/key/value: (16, 64, 256) fp32;  W_out: (64, 64);  b_out: (64,)
  scores[c,e,i,j] = q[c,e,i]*k[c,e,j]/8 ; attn = softmax_j ; out = attn @ v
  out.reshape(4096, 64) @ W_out.T + b_out  -> (4096, 64)

Sharding: the 16*64 = 1024 independent (c,e) attention problems are split
across 8 NeuronCores, 128 problems per core (pure data parallel), one
problem per SBUF partition.

Algorithm (separable polynomial softmax): scores factor as q_i * (k_j/8),
so with exp(s) ~= sum_d a_d s^d (Chebyshev fit on [-3,3]; observed
|s| <= 2.3):

  den_i = sum_j exp(q_i k_j/8) ~= sum_d (a_d 8^-d sum_j k_j^d) q_i^d
  num_i = sum_j exp(q_i k_j/8) v_j ~= sum_d (a_d 8^-d sum_j v_j k_j^d) q_i^d
  out_i = num_i / den_i

Engine mapping per core (no N^2 scores, no exp, no attn@v matvec):
  - k and v land via parallel DMA queues; the [k|k] chain multiplicand is
    a 0-stride broadcast view of the single k copy.
  - moment chain slot s = [k^{s+1} | v k^s] in bf16 ([128,512] DVE multiply
    per degree, 2x DVE mode), then 3 levels of bf16 pairwise folds (2x
    mode) and one small fp32-accumulate strided tensor_reduce giving
    M_1..M_6 | V_1..V_6 contiguously.  V_0 (dominant numerator term) is
    summed from fp32 v on the scalar engine via accum_out.
  - evaluation at the 256 q-points by even/odd quadratic Horner on fused
    [f|g] [128,512] tiles; high-degree steps in bf16, the last degree
    pair in fp32.  The linear terms b_{2s+1} q + b_{2s} run on ACT
    (per-partition scale+bias) in parallel with the DVE chain.
  - reciprocal_approx_fast + multiply -> attention out (bf16)
  - PE transpose + bf16 out-projection matmuls, fp32 bias add.
"""

import numpy as np
import ml_dtypes

_BF = ml_dtypes.bfloat16

_NCORES = 8
_C, _E, _N = 16, 64, 256
_PPC = _C * _E // _NCORES          # 128 problems (c,e rows) per core

_D = 6                             # polynomial degree (even)
# Chebyshev interpolant of exp on [-3, 3], monomial basis, degree 6.
_ACOEF = np.array([
    0.9999999999999991,
    1.0196584308848022,
    0.502354771405987,
    0.14944607281892283,
    0.03959829987271537,
    0.011993297734671608,
    0.001832198620043305,
], dtype=np.float64)

_cached = None


def _build_program():
    import concourse.bacc as bacc
    import concourse.mybir as mybir
    from concourse.tile import TileContext

    f32 = mybir.dt.float32
    bf16 = mybir.dt.bfloat16
    AF = mybir.ActivationFunctionType
    OP = mybir.AluOpType
    AX = mybir.AxisListType

    nc = bacc.Bacc(
        "TRN2", target_bir_lowering=False, debug=False, num_devices=_NCORES
    )

    qqv_d = nc.dram_tensor("qqv", [128, 768], f32, kind="ExternalInput").ap()
    kb1_d = nc.dram_tensor("kb1", [128, 256], bf16, kind="ExternalInput").ap()
    vb1_d = nc.dram_tensor("vb1", [128, 256], bf16, kind="ExternalInput").ap()
    cfb_d = nc.dram_tensor("cfb", [128, 208], f32, kind="ExternalInput").ap()
    cbb_d = nc.dram_tensor("cbb", [128, 192], bf16, kind="ExternalInput").ap()
    out_d = nc.dram_tensor("out", [128, 256], f32, kind="ExternalOutput").ap()

    D = _D
    NS = D + 1                      # chain slots 0..D
    H = D // 2                      # lin terms s = 0..H-1

    with TileContext(nc) as tc:
        with (
            tc.tile_pool(name="const", bufs=1) as cp,
            tc.tile_pool(name="horn", bufs=2) as hp,
            tc.tile_pool(name="ps", bufs=4, space="PSUM") as psp,
        ):
            qqv = cp.tile([128, 768], f32, tag="qqv")
            kp = cp.tile([128, 512 * NS], bf16, tag="kp")   # slots 0..D
            fb1 = cp.tile([128, 128 * (NS - 1) * 2], bf16, tag="fb1")
            fb2 = cp.tile([128, 64 * (NS - 1) * 2], bf16, tag="fb2")
            fb3 = cp.tile([128, 32 * (NS - 1) * 2], bf16, tag="fb3")
            cfb = cp.tile([128, 208], f32, tag="cfb")
            cbb = cp.tile([128, 192], bf16, tag="cbb")
            mom = cp.tile([128, 32], f32, tag="mom")
            bcf = cp.tile([128, 32], f32, tag="bcf")
            uu = cp.tile([128, 512], f32, tag="uu")
            uub = cp.tile([128, 512], bf16, tag="uub")
            linb = [
                cp.tile([128, 512], bf16, tag=f"linb{s}", name=f"linb{s}")
                for s in range(1, H)
            ]                        # linb[i] -> s = i+1 (bf16)
            lin0 = cp.tile([128, 512], f32, tag="lin0")
            scr = cp.tile([128, 256], f32, tag="scr")
            rf = cp.tile([128, 256], f32, tag="rf")
            o = cp.tile([128, 256], bf16, tag="o")
            oTs = [
                cp.tile([128, 128], bf16, tag=f"oTs{b}", name=f"oTs{b}")
                for b in (0, 1)
            ]
            final = cp.tile([128, 256], f32, tag="final")

            qq = qqv[:, 0:512]
            qt = qqv[:, 0:256]
            vf = qqv[:, 512:768]
            ac = cfb[:, 64:80]
            bb = cfb[:, 0:64]
            wt = cbb[:, 0:64]
            ones_row = cfb[0:1, 80:208]
            bias_row = cfb[0:1, 0:64]
            ident = cbb[:, 64:192]
            kb = kp[:, 0:256]                          # slot0 left = k
            # [k|k] as a 0-stride broadcast of the single k copy
            kkb = kb.rearrange("p (h j) -> p h j", h=1).broadcast_to(
                [128, 2, 256]
            )

            # parallel DMA dispatch across the three DMA-capable queues
            nc.sync.dma_start(kp[:, 0:256], kb1_d)     # slot0 left: k
            nc.scalar.dma_start(kp[:, 256:512], vb1_d)  # slot0 right: v
            nc.gpsimd.dma_start(qqv[:], qqv_d)
            nc.scalar.dma_start(cfb[:], cfb_d)
            nc.gpsimd.dma_start(cbb[:], cbb_d)

            # early, moment-independent work
            nc.scalar.activation(uu[:], qq, AF.Square)      # [u|u] = q^2
            nc.scalar.activation(uub[:], qq, AF.Square)     # bf16 copy
            nc.scalar.activation(                            # V_0 (fp32)
                scr[:], vf, AF.Copy, accum_out=mom[:, 19:20]
            )
            nc.vector.memset(bcf[:, 0:1], 256.0)            # b_0 = a_0*256

            # ---- moment chain (bf16): slot s = [k^{s+1} | v k^s] ----------
            for s in range(1, NS - 1):
                sv = kp[:, 512 * s : 512 * s + 512].rearrange(
                    "p (h j) -> p h j", h=2
                )
                pv = kp[:, 512 * (s - 1) : 512 * s].rearrange(
                    "p (h j) -> p h j", h=2
                )
                nc.vector.tensor_tensor(sv, pv, kkb, OP.mult)
            # last slot: only the v-chain half (v k^D) is needed
            nc.vector.tensor_tensor(
                kp[:, 512 * D + 256 : 512 * D + 512],
                kp[:, 512 * (D - 1) + 256 : 512 * D],
                kb,
                OP.mult,
            )

            # Two-group folds: high degrees (d=4..6) reduce first so the
            # ACT lin terms (which need b_5,b_4 and the t0 scales b_6) can
            # start ~1.5us earlier; low-degree folds fill the DVE bubble.
            v4 = kp[:, :].rearrange("p (s h j) -> p s h j", h=2, j=256)
            fA = cp.tile([128, 768], bf16, tag="fA")
            fA2 = cp.tile([128, 384], bf16, tag="fA2")
            fB = cp.tile([128, 768], bf16, tag="fB")
            fB2 = cp.tile([128, 384], bf16, tag="fB2")

            def emit_lin(dst, s):
                nc.scalar.activation(
                    dst[:, 0:256], qt, AF.Identity,
                    bias=bcf[:, 2 * s : 2 * s + 1],
                    scale=bcf[:, 2 * s + 1 : 2 * s + 2],
                )
                nc.scalar.activation(
                    dst[:, 256:512], qt, AF.Identity,
                    bias=bcf[:, 16 + 2 * s : 17 + 2 * s],
                    scale=bcf[:, 17 + 2 * s : 18 + 2 * s],
                )

            fAl = fA[:, 0:384].rearrange("p (s h j) -> p s h j", h=1, j=128)
            fAr = fA[:, 384:768].rearrange("p (s h j) -> p s h j", h=1, j=128)
            nc.vector.tensor_tensor(
                fAl[:], v4[:, 3:6, 0:1, 0:128], v4[:, 3:6, 0:1, 128:256], OP.add
            )
            nc.vector.tensor_tensor(
                fAr[:], v4[:, 4:7, 1:2, 0:128], v4[:, 4:7, 1:2, 128:256], OP.add
            )
            fAv = fA[:, :].rearrange("p (s j) -> p s j", j=128)
            fA2v = fA2[:, :].rearrange("p (s j) -> p s j", j=64)
            nc.vector.tensor_tensor(
                fA2v[:], fAv[:, :, 0:64], fAv[:, :, 64:128], OP.add
            )
            nc.vector.tensor_reduce(mom[:, 0:6], fA2v[:], AX.X, OP.add)
            nc.vector.tensor_tensor(
                bcf[:, 4:7], ac[:, 4:7], mom[:, 0:3], OP.mult
            )
            nc.vector.tensor_tensor(
                bcf[:, 20:23], ac[:, 4:7], mom[:, 3:6], OP.mult
            )

            emit_lin(linb[1], 2)                     # ACT, after combine A

            # t0 scales (need only b_6/c_6), then group-B folds fill the
            # DVE queue while ACT produces the s=2 lin terms
            t = hp.tile([128, 512], bf16, tag="t", name="t0")
            nc.vector.tensor_scalar(
                t[:, 0:256], uub[:, 0:256], bcf[:, D : D + 1], None, OP.mult
            )
            nc.vector.tensor_scalar(
                t[:, 256:512], uub[:, 256:512], bcf[:, 16 + D : 17 + D],
                None, OP.mult,
            )
            fBl = fB[:, 0:384].rearrange("p (s h j) -> p s h j", h=1, j=128)
            fBr = fB[:, 384:768].rearrange("p (s h j) -> p s h j", h=1, j=128)
            nc.vector.tensor_tensor(
                fBl[:], v4[:, 0:3, 0:1, 0:128], v4[:, 0:3, 0:1, 128:256], OP.add
            )
            nc.vector.tensor_tensor(
                fBr[:], v4[:, 1:4, 1:2, 0:128], v4[:, 1:4, 1:2, 128:256], OP.add
            )
            tn = hp.tile([128, 512], bf16, tag="t", name="a2")
            nc.vector.tensor_tensor(tn[:], t[:], linb[1][:], OP.add)
            tm = hp.tile([128, 512], bf16, tag="m", name="m2")
            nc.vector.tensor_tensor(tm[:], tn[:], uub[:], OP.mult)
            fBv = fB[:, :].rearrange("p (s j) -> p s j", j=128)
            fB2v = fB2[:, :].rearrange("p (s j) -> p s j", j=64)
            nc.vector.tensor_tensor(
                fB2v[:], fBv[:, :, 0:64], fBv[:, :, 64:128], OP.add
            )
            nc.vector.tensor_reduce(mom[:, 8:14], fB2v[:], AX.X, OP.add)
            nc.vector.tensor_tensor(
                bcf[:, 1:4], ac[:, 1:4], mom[:, 8:11], OP.mult
            )
            nc.vector.tensor_tensor(
                bcf[:, 16:17], ac[:, 0:1], mom[:, 19:20], OP.mult
            )
            nc.vector.tensor_tensor(
                bcf[:, 17:20], ac[:, 1:4], mom[:, 11:14], OP.mult
            )

            emit_lin(linb[0], 1)                     # ACT, after combine B
            emit_lin(lin0, 0)

            # last degree pair: one bf16 step then fp32 finish
            tn2 = hp.tile([128, 512], bf16, tag="t", name="a1")
            nc.vector.tensor_tensor(tn2[:], tm[:], linb[0][:], OP.add)
            m0 = hp.tile([128, 512], f32, tag="mf", name="m0")
            nc.vector.tensor_tensor(m0[:], tn2[:], uub[:], OP.mult)
            a0 = hp.tile([128, 512], f32, tag="tf", name="a0")
            nc.vector.tensor_tensor(a0[:], m0[:], lin0[:], OP.add)

            # ---- normalize: o = num / den (bf16 out) ----------------------
            nc.vector.reciprocal_approx_fast(rf[:], a0[:, 0:256])
            nc.vector.tensor_tensor(o[:], a0[:, 256:512], rf[:], OP.mult)

            # ---- out-projection: rows 4p+ii -------------------------------
            for b in range(2):
                tps = psp.tile([128, 128], bf16, tag="tps", name="tps")
                nc.tensor.transpose(tps[:], o[:, 128 * b : 128 * b + 128], ident)
                nc.vector.tensor_copy(oTs[b][:], tps[:])
            for blk in range(4):
                h = blk % 2
                pp = psp.tile([128, 64], f32, tag="pp", name="pp")
                nc.tensor.matmul(
                    pp[:],
                    oTs[blk // 2][64 * h : 64 * h + 64, :],
                    wt[64 * h : 64 * h + 64, :],
                    start=True,
                    stop=True,
                )
                nc.vector.tensor_tensor(
                    final[:, 64 * blk : 64 * blk + 64], pp[:], bb, OP.add
                )
                # chunked output DMA on rotating queues: each 32KB transfer
                # starts as soon as its block's bias add lands, overlapping
                # dispatch and transfer with the remaining projection work
                eng = (nc.sync, nc.scalar, nc.gpsimd, nc.sync)[blk]
                eng.dma_start(
                    out_d[:, 64 * blk : 64 * blk + 64],
                    final[:, 64 * blk : 64 * blk + 64],
                )

    nc.finalize()
    return nc


def _marshal(core, q2, k2, v2, cfb, cbb):
    lo = _PPC * core
    q = q2[lo : lo + _PPC]
    k = k2[lo : lo + _PPC]
    v = v2[lo : lo + _PPC]
    qqv = np.ascontiguousarray(np.concatenate([q, q, v], axis=1))
    kb1 = np.ascontiguousarray(k.astype(_BF))
    vb1 = np.ascontiguousarray(v.astype(_BF))
    return {"qqv": qqv, "kb1": kb1, "vb1": vb1, "cfb": cfb, "cbb": cbb}


def _shared_inputs(W_out, b_out):
    wt = np.tile(np.asarray(W_out, np.float32).T, (2, 1))
    bb = np.broadcast_to(np.asarray(b_out, np.float32), (128, 64))
    ident = np.eye(128, dtype=np.float32)
    acoef = (_ACOEF * (0.125 ** np.arange(_D + 1))).astype(np.float32)
    acz = np.zeros((128, 16), np.float32)
    acz[:, 0 : _D + 1] = acoef[None, :]
    ones = np.ones((128, 128), np.float32)
    cfb = np.ascontiguousarray(np.concatenate([bb, acz, ones], axis=1))
    cbb = np.ascontiguousarray(
        np.concatenate([wt, ident], axis=1).astype(_BF)
    )
    return cfb, cbb


def _in_maps_for_profile(np_inputs):
    q2 = np.asarray(np_inputs["query"], np.float32).reshape(_C * _E, _N)
    k2 = np.asarray(np_inputs["key"], np.float32).reshape(_C * _E, _N)
    v2 = np.asarray(np_inputs["value"], np.float32).reshape(_C * _E, _N)
    cfb, cbb = _shared_inputs(np_inputs["W_out"], np_inputs["b_out"])
    return [_marshal(m, q2, k2, v2, cfb, cbb) for m in range(_NCORES)]


def kernel(query, key, value, W_out, b_out):
    global _cached
    from concourse.bass_utils import run_bass_kernel_spmd

    if _cached is None:
        _cached = _build_program()
    nc = _cached

    q2 = np.asarray(query, np.float32).reshape(_C * _E, _N)
    k2 = np.asarray(key, np.float32).reshape(_C * _E, _N)
    v2 = np.asarray(value, np.float32).reshape(_C * _E, _N)
    cfb, cbb = _shared_inputs(W_out, b_out)

    in_maps = [_marshal(m, q2, k2, v2, cfb, cbb) for m in range(_NCORES)]
    res = run_bass_kernel_spmd(nc, in_maps, core_ids=list(range(_NCORES)))
    return np.concatenate(
        [res.results[m]["out"].reshape(4 * _PPC, _E) for m in range(_NCORES)], axis=0
    )


# revision 29
# speedup vs baseline: 1.0083x; 1.0083x over previous
"""Trainium2 Bass kernel for head_dim==1 cross-attention + out-projection.

Problem (hardcoded shapes):
  query/key/value: (16, 64, 256) fp32;  W_out: (64, 64);  b_out: (64,)
  scores[c,e,i,j] = q[c,e,i]*k[c,e,j]/8 ; attn = softmax_j ; out = attn @ v
  out.reshape(4096, 64) @ W_out.T + b_out  -> (4096, 64)

Sharding: the 16*64 = 1024 independent (c,e) attention problems are split
across 8 NeuronCores, 128 problems per core (pure data parallel), one
problem per SBUF partition.

Algorithm (separable polynomial softmax): scores factor as q_i * (k_j/8),
so with exp(s) ~= sum_d a_d s^d (Chebyshev fit on [-3,3]; observed
|s| <= 2.3):

  den_i = sum_j exp(q_i k_j/8) ~= sum_d (a_d 8^-d sum_j k_j^d) q_i^d
  num_i = sum_j exp(q_i k_j/8) v_j ~= sum_d (a_d 8^-d sum_j v_j k_j^d) q_i^d
  out_i = num_i / den_i

Engine mapping per core (no N^2 scores, no exp, no attn@v matvec):
  - k and v land via parallel DMA queues; the [k|k] chain multiplicand is
    a 0-stride broadcast view of the single k copy.
  - moment chain slot s = [k^{s+1} | v k^s] in bf16 ([128,512] DVE multiply
    per degree, 2x DVE mode), then 3 levels of bf16 pairwise folds (2x
    mode) and one small fp32-accumulate strided tensor_reduce giving
    M_1..M_6 | V_1..V_6 contiguously.  V_0 (dominant numerator term) is
    summed from fp32 v on the scalar engine via accum_out.
  - evaluation at the 256 q-points by even/odd quadratic Horner on fused
    [f|g] [128,512] tiles; high-degree steps in bf16, the last degree
    pair in fp32.  The linear terms b_{2s+1} q + b_{2s} run on ACT
    (per-partition scale+bias) in parallel with the DVE chain.
  - reciprocal_approx_fast + multiply -> attention out (bf16)
  - PE transpose + bf16 out-projection matmuls, fp32 bias add.
"""

import numpy as np
import ml_dtypes

_BF = ml_dtypes.bfloat16

_NCORES = 8
_C, _E, _N = 16, 64, 256
_PPC = _C * _E // _NCORES          # 128 problems (c,e rows) per core

_D = 6                             # polynomial degree (even)
# Chebyshev interpolant of exp on [-3, 3], monomial basis, degree 6.
_ACOEF = np.array([
    0.9999999999999991,
    1.0196584308848022,
    0.502354771405987,
    0.14944607281892283,
    0.03959829987271537,
    0.011993297734671608,
    0.001832198620043305,
], dtype=np.float64)

_cached = None


def _build_program():
    import concourse.bacc as bacc
    import concourse.mybir as mybir
    from concourse.tile import TileContext

    f32 = mybir.dt.float32
    bf16 = mybir.dt.bfloat16
    AF = mybir.ActivationFunctionType
    OP = mybir.AluOpType
    AX = mybir.AxisListType

    nc = bacc.Bacc(
        "TRN2", target_bir_lowering=False, debug=False, num_devices=_NCORES
    )

    qqv_d = nc.dram_tensor("qqv", [128, 768], f32, kind="ExternalInput").ap()
    kb1_d = nc.dram_tensor("kb1", [128, 256], bf16, kind="ExternalInput").ap()
    vb1_d = nc.dram_tensor("vb1", [128, 256], bf16, kind="ExternalInput").ap()
    cfb_d = nc.dram_tensor("cfb", [128, 208], f32, kind="ExternalInput").ap()
    cbb_d = nc.dram_tensor("cbb", [128, 192], bf16, kind="ExternalInput").ap()
    out_d = nc.dram_tensor("out", [128, 256], f32, kind="ExternalOutput").ap()

    D = _D
    NS = D + 1                      # chain slots 0..D
    H = D // 2                      # lin terms s = 0..H-1

    with TileContext(nc) as tc:
        with (
            tc.tile_pool(name="const", bufs=1) as cp,
            tc.tile_pool(name="horn", bufs=2) as hp,
            tc.tile_pool(name="ps", bufs=4, space="PSUM") as psp,
        ):
            qqv = cp.tile([128, 768], f32, tag="qqv")
            kp = cp.tile([128, 512 * NS], bf16, tag="kp")   # slots 0..D
            fb1 = cp.tile([128, 128 * (NS - 1) * 2], bf16, tag="fb1")
            fb2 = cp.tile([128, 64 * (NS - 1) * 2], bf16, tag="fb2")
            fb3 = cp.tile([128, 32 * (NS - 1) * 2], bf16, tag="fb3")
            cfb = cp.tile([128, 208], f32, tag="cfb")
            cbb = cp.tile([128, 192], bf16, tag="cbb")
            mom = cp.tile([128, 32], f32, tag="mom")
            bcf = cp.tile([128, 32], f32, tag="bcf")
            uu = cp.tile([128, 512], f32, tag="uu")
            uub = cp.tile([128, 512], bf16, tag="uub")
            linb = [
                cp.tile([128, 512], bf16, tag=f"linb{s}", name=f"linb{s}")
                for s in range(1, H)
            ]                        # linb[i] -> s = i+1 (bf16)
            lin0 = cp.tile([128, 512], f32, tag="lin0")
            scr = cp.tile([128, 256], f32, tag="scr")
            rf = cp.tile([128, 256], f32, tag="rf")
            o = cp.tile([128, 256], bf16, tag="o")
            oTs = [
                cp.tile([128, 128], bf16, tag=f"oTs{b}", name=f"oTs{b}")
                for b in (0, 1)
            ]
            final = cp.tile([128, 256], f32, tag="final")

            qq = qqv[:, 0:512]
            qt = qqv[:, 0:256]
            vf = qqv[:, 512:768]
            ac = cfb[:, 64:80]
            bb = cfb[:, 0:64]
            wt = cbb[:, 0:64]
            ones_row = cfb[0:1, 80:208]
            bias_row = cfb[0:1, 0:64]
            ident = cbb[:, 64:192]
            kb = kp[:, 0:256]                          # slot0 left = k
            # [k|k] as a 0-stride broadcast of the single k copy
            kkb = kb.rearrange("p (h j) -> p h j", h=1).broadcast_to(
                [128, 2, 256]
            )

            # parallel DMA dispatch across the three DMA-capable queues
            nc.sync.dma_start(kp[:, 0:256], kb1_d)     # slot0 left: k
            nc.scalar.dma_start(kp[:, 256:512], vb1_d)  # slot0 right: v
            nc.gpsimd.dma_start(qqv[:], qqv_d)
            nc.scalar.dma_start(cfb[:], cfb_d)
            nc.gpsimd.dma_start(cbb[:], cbb_d)

            # early, moment-independent work
            nc.scalar.activation(uu[:], qq, AF.Square)      # [u|u] = q^2
            nc.scalar.activation(uub[:], qq, AF.Square)     # bf16 copy
            nc.scalar.activation(                            # V_0 (fp32)
                scr[:], vf, AF.Copy, accum_out=mom[:, 19:20]
            )
            nc.vector.memset(bcf[:, 0:1], 256.0)            # b_0 = a_0*256

            # ---- moment chain (bf16): slot s = [k^{s+1} | v k^s] ----------
            for s in range(1, NS - 1):
                sv = kp[:, 512 * s : 512 * s + 512].rearrange(
                    "p (h j) -> p h j", h=2
                )
                pv = kp[:, 512 * (s - 1) : 512 * s].rearrange(
                    "p (h j) -> p h j", h=2
                )
                nc.vector.tensor_tensor(sv, pv, kkb, OP.mult)
            # last slot: only the v-chain half (v k^D) is needed
            nc.vector.tensor_tensor(
                kp[:, 512 * D + 256 : 512 * D + 512],
                kp[:, 512 * (D - 1) + 256 : 512 * D],
                kb,
                OP.mult,
            )

            # Two-group folds: high degrees (d=4..6) reduce first so the
            # ACT lin terms (which need b_5,b_4 and the t0 scales b_6) can
            # start ~1.5us earlier; low-degree folds fill the DVE bubble.
            v4 = kp[:, :].rearrange("p (s h j) -> p s h j", h=2, j=256)
            fA = cp.tile([128, 768], bf16, tag="fA")
            fA2 = cp.tile([128, 384], bf16, tag="fA2")
            fB = cp.tile([128, 768], bf16, tag="fB")
            fB2 = cp.tile([128, 384], bf16, tag="fB2")

            def emit_lin(dst, s):
                nc.scalar.activation(
                    dst[:, 0:256], qt, AF.Identity,
                    bias=bcf[:, 2 * s : 2 * s + 1],
                    scale=bcf[:, 2 * s + 1 : 2 * s + 2],
                )
                nc.scalar.activation(
                    dst[:, 256:512], qt, AF.Identity,
                    bias=bcf[:, 16 + 2 * s : 17 + 2 * s],
                    scale=bcf[:, 17 + 2 * s : 18 + 2 * s],
                )

            fAl = fA[:, 0:384].rearrange("p (s h j) -> p s h j", h=1, j=128)
            fAr = fA[:, 384:768].rearrange("p (s h j) -> p s h j", h=1, j=128)
            nc.vector.tensor_tensor(
                fAl[:], v4[:, 3:6, 0:1, 0:128], v4[:, 3:6, 0:1, 128:256], OP.add
            )
            nc.vector.tensor_tensor(
                fAr[:], v4[:, 4:7, 1:2, 0:128], v4[:, 4:7, 1:2, 128:256], OP.add
            )
            fAv = fA[:, :].rearrange("p (s j) -> p s j", j=128)
            fA2v = fA2[:, :].rearrange("p (s j) -> p s j", j=64)
            nc.vector.tensor_tensor(
                fA2v[:], fAv[:, :, 0:64], fAv[:, :, 64:128], OP.add
            )
            nc.vector.tensor_reduce(mom[:, 0:6], fA2v[:], AX.X, OP.add)
            nc.vector.tensor_tensor(
                bcf[:, 4:7], ac[:, 4:7], mom[:, 0:3], OP.mult
            )
            nc.vector.tensor_tensor(
                bcf[:, 20:23], ac[:, 4:7], mom[:, 3:6], OP.mult
            )

            emit_lin(linb[1], 2)                     # ACT, after combine A

            # t0 scales (need only b_6/c_6), then group-B folds fill the
            # DVE queue while ACT produces the s=2 lin terms
            t = hp.tile([128, 512], bf16, tag="t", name="t0")
            nc.vector.tensor_scalar(
                t[:, 0:256], uub[:, 0:256], bcf[:, D : D + 1], None, OP.mult
            )
            nc.vector.tensor_scalar(
                t[:, 256:512], uub[:, 256:512], bcf[:, 16 + D : 17 + D],
                None, OP.mult,
            )
            fBl = fB[:, 0:384].rearrange("p (s h j) -> p s h j", h=1, j=128)
            fBr = fB[:, 384:768].rearrange("p (s h j) -> p s h j", h=1, j=128)
            nc.vector.tensor_tensor(
                fBl[:], v4[:, 0:3, 0:1, 0:128], v4[:, 0:3, 0:1, 128:256], OP.add
            )
            nc.vector.tensor_tensor(
                fBr[:], v4[:, 1:4, 1:2, 0:128], v4[:, 1:4, 1:2, 128:256], OP.add
            )
            tn = hp.tile([128, 512], bf16, tag="t", name="a2")
            nc.vector.tensor_tensor(tn[:], t[:], linb[1][:], OP.add)
            tm = hp.tile([128, 512], bf16, tag="m", name="m2")
            nc.vector.tensor_tensor(tm[:], tn[:], uub[:], OP.mult)
            fBv = fB[:, :].rearrange("p (s j) -> p s j", j=128)
            fB2v = fB2[:, :].rearrange("p (s j) -> p s j", j=64)
            nc.vector.tensor_tensor(
                fB2v[:], fBv[:, :, 0:64], fBv[:, :, 64:128], OP.add
            )
            nc.vector.tensor_reduce(mom[:, 8:14], fB2v[:], AX.X, OP.add)
            nc.vector.tensor_tensor(
                bcf[:, 1:4], ac[:, 1:4], mom[:, 8:11], OP.mult
            )
            nc.vector.tensor_tensor(
                bcf[:, 16:17], ac[:, 0:1], mom[:, 19:20], OP.mult
            )
            nc.vector.tensor_tensor(
                bcf[:, 17:20], ac[:, 1:4], mom[:, 11:14], OP.mult
            )

            emit_lin(linb[0], 1)                     # ACT, after combine B
            emit_lin(lin0, 0)

            # last degree pair: one bf16 step then fp32 finish
            tn2 = hp.tile([128, 512], bf16, tag="t", name="a1")
            nc.vector.tensor_tensor(tn2[:], tm[:], linb[0][:], OP.add)
            m0 = hp.tile([128, 512], f32, tag="mf", name="m0")
            nc.vector.tensor_tensor(m0[:], tn2[:], uub[:], OP.mult)
            a0 = hp.tile([128, 512], f32, tag="tf", name="a0")
            nc.vector.tensor_tensor(a0[:], m0[:], lin0[:], OP.add)

            # ---- normalize: o = num / den (bf16 out) ----------------------
            nc.vector.reciprocal_approx_fast(rf[:], a0[:, 0:256])
            nc.vector.tensor_tensor(o[:], a0[:, 256:512], rf[:], OP.mult)

            # ---- out-projection: rows 4p+ii -------------------------------
            for b in range(2):
                tps = psp.tile([128, 128], bf16, tag="tps", name="tps")
                nc.tensor.transpose(tps[:], o[:, 128 * b : 128 * b + 128], ident)
                nc.vector.tensor_copy(oTs[b][:], tps[:])
            for blk in range(4):
                h = blk % 2
                pp = psp.tile([128, 64], f32, tag="pp", name="pp")
                nc.tensor.matmul(
                    pp[:],
                    oTs[blk // 2][64 * h : 64 * h + 64, :],
                    wt[64 * h : 64 * h + 64, :],
                    start=True,
                    stop=True,
                )
                nc.vector.tensor_tensor(
                    final[:, 64 * blk : 64 * blk + 64], pp[:], bb, OP.add
                )
            nc.sync.dma_start(out_d, final[:])

    nc.finalize()
    return nc


def _marshal(core, q2, k2, v2, cfb, cbb):
    lo = _PPC * core
    q = q2[lo : lo + _PPC]
    k = k2[lo : lo + _PPC]
    v = v2[lo : lo + _PPC]
    qqv = np.ascontiguousarray(np.concatenate([q, q, v], axis=1))
    kb1 = np.ascontiguousarray(k.astype(_BF))
    vb1 = np.ascontiguousarray(v.astype(_BF))
    return {"qqv": qqv, "kb1": kb1, "vb1": vb1, "cfb": cfb, "cbb": cbb}


def _shared_inputs(W_out, b_out):
    wt = np.tile(np.asarray(W_out, np.float32).T, (2, 1))
    bb = np.broadcast_to(np.asarray(b_out, np.float32), (128, 64))
    ident = np.eye(128, dtype=np.float32)
    acoef = (_ACOEF * (0.125 ** np.arange(_D + 1))).astype(np.float32)
    acz = np.zeros((128, 16), np.float32)
    acz[:, 0 : _D + 1] = acoef[None, :]
    ones = np.ones((128, 128), np.float32)
    cfb = np.ascontiguousarray(np.concatenate([bb, acz, ones], axis=1))
    cbb = np.ascontiguousarray(
        np.concatenate([wt, ident], axis=1).astype(_BF)
    )
    return cfb, cbb


def _in_maps_for_profile(np_inputs):
    q2 = np.asarray(np_inputs["query"], np.float32).reshape(_C * _E, _N)
    k2 = np.asarray(np_inputs["key"], np.float32).reshape(_C * _E, _N)
    v2 = np.asarray(np_inputs["value"], np.float32).reshape(_C * _E, _N)
    cfb, cbb = _shared_inputs(np_inputs["W_out"], np_inputs["b_out"])
    return [_marshal(m, q2, k2, v2, cfb, cbb) for m in range(_NCORES)]


def kernel(query, key, value, W_out, b_out):
    global _cached
    from concourse.bass_utils import run_bass_kernel_spmd

    if _cached is None:
        _cached = _build_program()
    nc = _cached

    q2 = np.asarray(query, np.float32).reshape(_C * _E, _N)
    k2 = np.asarray(key, np.float32).reshape(_C * _E, _N)
    v2 = np.asarray(value, np.float32).reshape(_C * _E, _N)
    cfb, cbb = _shared_inputs(W_out, b_out)

    in_maps = [_marshal(m, q2, k2, v2, cfb, cbb) for m in range(_NCORES)]
    res = run_bass_kernel_spmd(nc, in_maps, core_ids=list(range(_NCORES)))
    return np.concatenate(
        [res.results[m]["out"].reshape(4 * _PPC, _E) for m in range(_NCORES)], axis=0
    )
